# revision 40
# baseline (speedup 1.0000x reference)
"""GGML Q8_0 fused dequant + mat-vec kernel for Trainium2 (8 NeuronCores).

out[b, o] = sum_{k} x[b, k] * scales[o, k//32] * q[o, k] + bias[o]
  x: [1, 4096] f32, q: [14336, 4096] int32 (int8 values), scales: [14336, 128] f32,
  bias: [14336] f32 -> out [1, 14336] f32

Sharding: row-parallel (out_features) across 8 cores; x replicated.

Strategy (memory-roofline): stream q as int8 (1 B/elem, 7.2 MB/core/pass)
in a host-pretransposed SBUF image [128 k-partitions, 32 chunks x 1792 rows].
Per k-chunk on device:
  1. DVE/ACT convert int8 -> fp16 (split 20/12 chunks to balance engine rates)
  2. TensorE: psum[128 blocks, 1792] += Xmask_c[128,128].T @ w16_c[128, 512-tiles]
     where Xmask_c[p, b] = x[128c+p] iff b == 4c + p//32 (block-masked x), so
     psum accumulates per-(Q8 block, row) partial dot products.
Then one DVE multiply by scales^T [128 blocks, 1792] and a ones[128,1] matmul
reduces over blocks -> out[1, 1792].  Bias is added on host after the gather.
Per-core per-pass HBM ~7.6 MB -> ~21 us; PE ~25 us paces the pipeline.
"""

import sys

import numpy as np

if "/opt/trn_rl_repo" not in sys.path:
    sys.path.insert(0, "/opt/trn_rl_repo")

OUT_F = 14336
IN_F = 4096
BLOCK = 32
NB = IN_F // BLOCK  # 128 blocks per row
N_CORES = 8
ROWS = OUT_F // N_CORES  # 1792 rows per core
P = 128  # partitions
NCHUNK = IN_F // P  # 32 k-chunks
WCOLS = NCHUNK * ROWS  # 57344 sbuf columns
OTILE = 512  # psum-bank-aligned output tiles: 512,512,512,256
OSPLITS = [(0, 512), (512, 512), (1024, 512), (1536, 256)]
NPAIRS = NCHUNK // 2  # int8->fp16 conversion granularity: 2 chunks per op
# pair -> convert engine, weighted by rates (DVE ~1.92, ACT ~1.2
# Gelem/s/partition; GPSIMD's software copy measured far too slow on HW):
# 10 DVE / 6 ACT
CONV_ENG = (["dve", "act", "dve", "act", "dve", "act", "dve", "dve"] * 2)

_NC_CACHE = {}


def _patch_tile_exit_drain():
    """Split the TileContext exit-drain sem waits across 1-wait NOPs.

    The walrus in this container lowers SP CTRL (NoOp/Drain) instructions
    with at most ONE sync-wait command; Tile's kernel-tail drain attaches a
    wait per live semaphore to a single instruction, which fails codegen
    with "Too many sync wait commands".  Redistribute the waits across a
    chain of SP NOPs (sequential on the SP stream, so ordering semantics
    are preserved) before the drain.
    """
    import concourse.mybir as mybir
    import concourse.tile as tile

    if getattr(tile.TileContext, "_ant_drain_patch", False):
        return

    def _drain_and_barrier(self, tick_clock, wait_clock):
        nc = self.nc
        carrier = nc.sync.nop(nofuse=True)
        wait_clock.add_sem_waits(
            carrier.ins, tile.ScopedClock({None: tick_clock.global_clock}))
        si = carrier.ins.sync_info
        waits = list(si.on_wait) if si is not None else []
        if len(waits) > 1:
            carrier.ins.sync_info = mybir.SyncInfo(
                on_wait=waits[:1], on_update=list(si.on_update))
            for i in range(1, len(waits)):
                extra = nc.sync.nop(nofuse=True)
                extra.ins.sync_info = mybir.SyncInfo(
                    on_wait=waits[i:i + 1], on_update=[])
        nc.sync.drain()
        nc.all_engine_barrier()
        assert self.sems is not None
        popped = nc._tile_sem_poison_stack.pop()
        assert popped is self._sem_poison
        nc.clear_and_free_semaphores(list(self.sems.allocated().values()))
        nc.all_engine_barrier()

    tile.TileContext._drain_and_barrier = _drain_and_barrier
    tile.TileContext._ant_drain_patch = True


def _legalize_sync_waits(nc):
    """Split multi-wait instructions for a walrus that encodes one sync wait.

    Tile's semaphore assignment may attach several sem waits to one
    instruction; this walrus build rejects >1 ("Too many sync wait
    commands").  Hoist all but the last wait onto NoOp instructions injected
    just before the instruction on the same engine (engine streams execute
    in order, so the wait semantics are unchanged).
    """
    import concourse.mybir as mybir

    n_split = 0
    for f in nc.m.functions:
        for bb in f.blocks:
            il = bb.instructions
            if not any(
                ins.sync_info is not None and len(ins.sync_info.on_wait) > 1
                for ins in il
            ):
                continue
            new = []
            for ins in il:
                si = ins.sync_info
                if si is not None and len(si.on_wait) > 1:
                    waits = list(si.on_wait)
                    for w in waits[:-1]:
                        nop = mybir.InstNoOp(
                            name=f"I-waitnop-{nc.next_id()}", ins=[], outs=[])
                        nop.engine = ins.engine
                        nop.sync_info = mybir.SyncInfo(
                            on_wait=[w], on_update=[])
                        nc.register_instruction(nop, overwrite=True)
                        new.append(nop)
                        n_split += 1
                    ins.sync_info = mybir.SyncInfo(
                        on_wait=[waits[-1]], on_update=list(si.on_update))
                new.append(ins)
            il[:] = new
    return n_split


def _build_nc(passes=1):
    """Build the per-core Bass program.

    passes>1 repeats the whole (idempotent) computation inside one NEFF —
    used only by the benchmark harness to measure steady-state per-pass
    device time by differencing wall clocks of two NEFF variants.
    """
    if passes in _NC_CACHE:
        return _NC_CACHE[passes]

    import concourse.bass as bass
    import concourse.mybir as mybir
    import concourse.tile as tile

    _patch_tile_exit_drain()

    f32 = mybir.dt.float32
    f16 = mybir.dt.float16
    i8 = mybir.dt.int8

    nc = bass.Bass("TRN2", target_bir_lowering=False, debug=False,
                   num_devices=N_CORES)

    q_d = nc.dram_tensor("q8", [P, WCOLS], i8, kind="ExternalInput").ap()
    xs_d = nc.dram_tensor("xmask", [P, NCHUNK * 32], f16,
                          kind="ExternalInput").ap()
    sc_d = nc.dram_tensor("scT", [P, ROWS], f16, kind="ExternalInput").ap()
    out_d = nc.dram_tensor("out", [1, ROWS], f32, kind="ExternalOutput").ap()

    with nc.allow_low_precision("fp16 weights/partials; f32 psum accum"):
        with tile.TileContext(nc) as tc:
            with (
                tc.tile_pool(name="const", bufs=1) as constp,
                tc.tile_pool(name="stage", bufs=2) as stagep,
                tc.tile_pool(name="w16", bufs=5) as w16p,
                tc.tile_pool(name="s2", bufs=2) as s2p,
                tc.tile_pool(name="ob", bufs=2) as obp,
                tc.tile_pool(name="ps", bufs=2, space="PSUM") as psp,
            ):
                xs = constp.tile([P, NCHUNK * 32], f16, name="xs")
                nc.sync.dma_start(out=xs, in_=xs_d)
                ones = constp.tile([P, 1], f16, name="ones")
                nc.vector.memset(ones, 1.0)
                sct = constp.tile([P, ROWS], f16, name="sct")
                nc.sync.dma_start(out=sct, in_=sc_d)

                for _rep in range(passes):
                    # [128, 2048] = exactly 4 psum banks; cols 0-1791 hold the
                    # per-(block,row) partials, cols 1792-2047 are the
                    # block-reduce scratch.  bufs=2 -> passes ping-pong the
                    # two 4-bank halves: no WAR stall at the pass boundary.
                    pp = psp.tile([P, 2048], f32, name="pp")
                    # one whole-stream DMA (7.2 MiB contiguous per partition
                    # line) for best HBM efficiency; stage bufs=2 lets pass
                    # p+1's DMA stream while pass p converts/matmuls.
                    stg = stagep.tile([P, WCOLS], i8, name="stg")
                    nc.sync.dma_start(out=stg, in_=q_d)
                    for pr in range(NPAIRS):
                        w16 = w16p.tile([P, 2 * ROWS], f16, name="w16")
                        src = stg[:, pr * 2 * ROWS:(pr + 1) * 2 * ROWS]
                        eng = CONV_ENG[pr]
                        if eng == "dve":
                            nc.vector.tensor_copy(w16, src)
                        elif eng == "act":
                            nc.scalar.copy(w16, src)
                        else:
                            nc.gpsimd.tensor_copy(w16, src)
                        for ci in range(2):
                            c = pr * 2 + ci
                            # col-group grp holds chunk c's 32-col stationary;
                            # consecutive chunks land in different 32-col PE
                            # groups, so their matmuls run concurrently in
                            # the array.  Block 4c+j sits at psum partition
                            # 32*(c%4) + 4*(c//4) + j (host permutes scales
                            # to match; the ones-reduce is order-invariant).
                            grp, rnd = c % 4, c // 4
                            lhs = xs[:, c * 32:(c + 1) * 32]
                            wsl = w16[:, ci * ROWS:(ci + 1) * ROWS]
                            for off, sz in OSPLITS:
                                nc.tensor.matmul(
                                    pp[32 * grp:32 * (grp + 1), off:off + sz],
                                    lhs,
                                    wsl[:, off:off + sz],
                                    start=(rnd == 0),
                                    stop=(rnd == NCHUNK // 4 - 1),
                                    tile_position=(0, 32 * grp),
                                )
                    s2 = s2p.tile([P, ROWS], f16, name="s2")
                    nc.vector.tensor_mul(s2, pp[:, :ROWS], sct)
                    osb = obp.tile([1, ROWS], f32, name="osb")
                    p3 = pp[0:1, ROWS:ROWS + 256]  # scratch in pp's tail bank
                    for off in range(0, ROWS, 256):
                        nc.tensor.matmul(
                            p3, ones, s2[:, off:off + 256],
                            start=True, stop=True)
                        nc.scalar.copy(osb[:, off:off + 256], p3)
                    # out-DMA on the ACT HWDGE ring: its sem-wait must not
                    # block the SP ring that streams weights.
                    nc.scalar.dma_start(out=out_d, in_=osb)

    _legalize_sync_waits(nc)
    _NC_CACHE[passes] = nc
    return nc


def _make_in_maps(x, q, scales, bias):
    x = np.asarray(x, dtype=np.float32).reshape(1, IN_F)
    q = np.asarray(q, dtype=np.int32).reshape(OUT_F, IN_F)
    scales = np.asarray(scales, dtype=np.float32).reshape(OUT_F, NB)

    # block-masked stationary x, 32 columns per chunk (PE col-group tiling):
    # xmask[p, c*32 + m] = x[c*128+p] iff m == 4*(c//4) + p//32
    x16 = x.reshape(IN_F).astype(np.float16)
    xs = np.zeros((P, NCHUNK, 32), dtype=np.float16)
    p_idx = np.arange(P)
    b_loc = p_idx // BLOCK  # 0..3
    for c in range(NCHUNK):
        xs[p_idx, c, 4 * (c // 4) + b_loc] = x16[c * P + p_idx]
    xs = np.ascontiguousarray(xs.reshape(P, NCHUNK * 32))

    # block b = 4c+j lands at psum partition pi(b) = 32*(c%4) + 4*(c//4) + j
    b_all = np.arange(NB)
    c_all, j_all = b_all // 4, b_all % 4
    pi = 32 * (c_all % 4) + 4 * (c_all // 4) + j_all

    in_maps = []
    for core in range(N_CORES):
        r0 = core * ROWS
        # SBUF image: H8[p, c*ROWS + o] = q[r0+o, c*128 + p]
        h8 = np.ascontiguousarray(
            q[r0:r0 + ROWS].T.reshape(NCHUNK, P, ROWS).transpose(1, 0, 2)
            .reshape(P, WCOLS).astype(np.int8))
        sct = np.empty((NB, ROWS), dtype=np.float16)
        sct[pi] = scales[r0:r0 + ROWS].T.astype(np.float16)
        sct = np.ascontiguousarray(sct)
        in_maps.append({"q8": h8, "xmask": xs, "scT": sct})
    return in_maps


def _gather(results, bias):
    parts = [np.asarray(results[c]["out"], dtype=np.float32).reshape(ROWS)
             for c in range(N_CORES)]
    out = np.concatenate(parts) + np.asarray(bias, dtype=np.float32)
    return out.reshape(1, OUT_F).astype(np.float32)


def kernel(x, q, scales, bias):
    from concourse.bass_utils import run_bass_kernel_spmd

    nc = _build_nc()
    in_maps = _make_in_maps(x, q, scales, bias)
    res = run_bass_kernel_spmd(nc, in_maps, list(range(N_CORES)))
    return _gather(res.results, bias)
